# revision 16
# baseline (speedup 1.0000x reference)
"""Single-head causal attention (B=4, T=2048, C=1024, H=64) on 8 TRN2 NeuronCores.

Sharding: batch b -> core pair (2b, 2b+1); core parity p owns interleaved
128-row key tiles {2m+p}.  Each core projects q for ALL 2048 queries and k,v
for its own 1024 keys, computes causal scores^T -> exp -> stair mask ->
wei@[v|1] partials for all queries vs its own keys.  Host adds pair partials
and normalizes (denominator = ones-column of the augmented v matmul).

Query-GROUP pipelined schedule: x arrives in 4 column groups (one per local
512-query tile, own+peer halves); per group g the PE runs projections ->
scores(g) while the scalar engine exps the previous pairs and PV(g-1)
interleaves.  The exp chain (the serial ACT bottleneck, ~8.5us) starts as
soon as group 0 is projected instead of after the full 4.4MB x load.

Data layout (all bf16, host pre-cast):
 - xT dram [128, 16384]: 8 slabs of 2048 cols, slab (g,h) = group g,
   h=0 own / h=1 peer query columns in pair-swapped order [t(2g+1)|t(2g)],
   chunk-major over C (8 chunks of 128 C-rows x 256 cols).
 - per group g, chunk c: qk mm (M=128, [wq|wk]) over own cols; qp (M=64,
   col group 0) and vv (M=64, col group 1) run CONCURRENTLY as column-tiled
   mms over peer / own cols.
 - q_sb local col order per 512 block: [own-odd | peer-odd | own-even |
   peer-even]; k_sb / vT_sb storage slot s holds own tile s^1.
 - k (and q) duplicated in both 64-row partition halves so the two
   row-tiled score mms of a slot pair run concurrently.
 - causal trimming + stair masks identical to the monolithic version.
"""

import os
import sys

sys.path.insert(0, "/opt/trn_rl_repo")

import numpy as np
import ml_dtypes

B, T, C, H = 4, 2048, 1024, 64
QT = 4
SCALE = float(C) ** -0.5
NWARM = 4

_COMPILED = None
LAST_EXEC_NS = None
LAST_RESULTS = None


def _build_nc():
    import concourse.bass as bass_mod
    import concourse.mybir as mybir
    import concourse.tile as tile
    from concourse import bacc
    from contextlib import ExitStack

    fp32 = mybir.dt.float32
    bf16 = mybir.dt.bfloat16
    fp8 = mybir.dt.float8e4

    nc = bacc.Bacc(
        "TRN2",
        target_bir_lowering=False,
        debug=False,
        num_devices=8,
        detect_race_conditions=True,
    )
    xT = nc.declare_dram_parameter("xT", [128, 16384], bf16, isOutput=False)
    wqkk = nc.declare_dram_parameter("wqkk", [128, 8, 128], bf16, isOutput=False)
    wv = nc.declare_dram_parameter("wv", [128, 8, 64], bf16, isOutput=False)
    maskneg = nc.declare_dram_parameter("maskneg", [128, 512], bf16, isOutput=False)
    out_ext = nc.declare_dram_parameter("out", [H + 1, T], fp32, isOutput=True)
    junkd = nc.declare_dram_parameter("junkd", [128, 16], bf16, isOutput=True)

    with ExitStack() as ctx:
        tc = ctx.enter_context(tile.TileContext(nc))
        persist = ctx.enter_context(tc.tile_pool(name="persist", bufs=1))
        weipool = ctx.enter_context(tc.tile_pool(name="wei", bufs=3))
        outpool = ctx.enter_context(tc.tile_pool(name="outp", bufs=2))

        xT_sb = persist.tile([128, 16384], bf16, tag="xT_sb")
        wqkk_sb = persist.tile([128, 8, 128], bf16, tag="wqkk_sb")
        wv_sb = persist.tile([128, 8, 64], bf16, tag="wv_sb")
        maskneg_sb = persist.tile([128, 512], bf16, tag="maskneg_sb")
        q_sb = persist.tile([128, T], bf16, tag="q_sb")
        k_sb = persist.tile([128, 512], bf16, tag="k_sb")
        vT_sb = persist.tile([64, 1024], bf16, tag="vT_sb")
        v_sb = persist.tile([128, 8, H + 1], bf16, tag="v_sb")
        ident = persist.tile([128, 128], bf16, tag="ident")
        scratch = persist.tile([128, 512], bf16, tag="scratch")
        warm_tok = persist.tile([1, 8], fp32, tag="warm_tok")
        act_tok = persist.tile([1, 8], bf16, tag="act_tok")
        junk_sb = persist.tile([128, 16], bf16, tag="junk_sb")

        # ---- gpsimd: constants first (they gate the PE warmup), DMAs none.
        nc.gpsimd.memset(scratch[:], 0.0)
        nc.gpsimd.memset(ident[:], 0.0)
        nc.gpsimd.affine_select(
            out=ident[:],
            in_=ident[:],
            compare_op=mybir.AluOpType.not_equal,
            fill=1.0,
            base=0,
            pattern=[[-1, 128]],
            channel_multiplier=1,
        )
        nc.gpsimd.memset(v_sb[:, :, H : H + 1], 1.0)

        weis = [
            weipool.tile([128, (2 * g + 2) * 512], bf16, tag="wei", name=f"wei{g}")
            for g in range(QT)
        ]

        # ---- loads: all of x on the sync HWDGE ring in consumption order
        # (group-major, own/peer chunk-halves interleaved so proj(g) can
        # start on chunks 0-3); weights on scalar (land first, then the
        # scalar queue is free for the exp chain).
        def slab(g, h):
            return (2 * g + h) * 2048

        # x: a DMA ring round-robins packets across ALL queued transfers
        # (they finish together), so sequential group arrival requires
        # chaining each issue on the previous group's landing / on exp
        # progress (the serial bottleneck).  4 sub-transfers per slab keep
        # ~8 engines busy (~280 GB/s) while exactly one group is in flight.
        def tail_ap(b0):
            # tails of the first two sub-transfers: the NEXT group's issue
            # fires when the previous group is ~half landed (overlap, no
            # ring dead-air at group boundaries)
            s = xT_sb[:, b0 + 511 : b0 + 512]
            return bass_mod.AP(
                tensor=s.tensor, offset=s.offset, ap=[s.ap[0], [512, 2]]
            )

        def x_issue(ring, g, h):
            b0 = slab(g, h)
            for s4 in range(4):
                ring.dma_start(
                    out=xT_sb[:, b0 + s4 * 512 : b0 + (s4 + 1) * 512],
                    in_=xT[:, b0 + s4 * 512 : b0 + (s4 + 1) * 512],
                )

        nc.gpsimd.dma_start(out=wqkk_sb[:], in_=wqkk[:])
        nc.sync.dma_start(out=wv_sb[:], in_=wv[:])
        x_issue(nc.gpsimd, 0, 0)
        x_issue(nc.sync, 0, 1)
        nc.sync.dma_start(out=maskneg_sb[:], in_=maskneg[:])
        # g1 gated on g0 arrival
        nc.gpsimd.tensor_copy(junk_sb[:, 0:2], tail_ap(slab(0, 0)))
        nc.sync.dma_start(out=junkd[:, 0:2], in_=tail_ap(slab(0, 1)))
        x_issue(nc.gpsimd, 1, 0)
        x_issue(nc.sync, 1, 1)
        # g2 gated on g1 arrival, g3 on g2 arrival
        nc.gpsimd.tensor_copy(junk_sb[:, 4:6], tail_ap(slab(1, 0)))
        nc.sync.dma_start(out=junkd[:, 4:6], in_=tail_ap(slab(1, 1)))
        x_issue(nc.gpsimd, 2, 0)
        x_issue(nc.sync, 2, 1)
        nc.gpsimd.tensor_copy(junk_sb[:, 8:10], tail_ap(slab(2, 0)))
        nc.sync.dma_start(out=junkd[:, 8:10], in_=tail_ap(slab(2, 1)))
        x_issue(nc.gpsimd, 3, 0)
        x_issue(nc.sync, 3, 1)

        # ---- PE pre-warm (gated only on gpsimd constants) + Exp spline
        # table preload on ACT.
        with tc.tile_pool(name="ps_warm", bufs=1, space="PSUM") as ps_warm:
            wps = ps_warm.tile([128, 512], fp32, tag="warm", name="warm_ps")
            for i in range(NWARM):
                nc.tensor.matmul(
                    out=wps[:],
                    lhsT=ident[:],
                    rhs=scratch[:],
                    start=(i == 0),
                    stop=(i == NWARM - 1),
                    skip_group_check=True,
                )
            nc.vector.tensor_copy(warm_tok[0:1, 0:8], wps[0:1, 0:8])
        nc.scalar.activation(
            out=act_tok[0:1, 0:8],
            in_=scratch[0:1, 0:8],
            func=mybir.ActivationFunctionType.Exp,
        )

        # ---- per-group projections ----
        ps_qk = ctx.enter_context(tc.tile_pool(name="ps_qk", bufs=1, space="PSUM"))
        ps_pv2 = ctx.enter_context(tc.tile_pool(name="ps_pv2", bufs=1, space="PSUM"))
        ps_pair = ctx.enter_context(tc.tile_pool(name="ps_pair", bufs=2, space="PSUM"))
        ps_pv = ctx.enter_context(tc.tile_pool(name="ps_pv", bufs=1, space="PSUM"))

        def scatter2(src, dst_base, dst_coloff):
            # src [64, 256] psum (two 128-col blocks) -> q_sb cols
            # {dst_coloff, dst_coloff+256} (one strided copy).
            s = src[:, 0:1]
            src_ap = bass_mod.AP(
                tensor=s.tensor, offset=s.offset, ap=[s.ap[0], [128, 2], [1, 128]]
            )
            d = dst_base[:, dst_coloff : dst_coloff + 1]
            dst_ap = bass_mod.AP(
                tensor=d.tensor, offset=d.offset, ap=[d.ap[0], [256, 2], [1, 128]]
            )
            nc.vector.tensor_copy(dst_ap, src_ap)

        def proj_group(g):
            qk_ps = ps_qk.tile([128, 256], fp32, tag="qk", name=f"qk{g}")
            pv2_ps = ps_pv2.tile([128, 512], fp32, tag="pv2", name=f"pv2{g}")
            own0 = slab(g, 0)
            peer0 = slab(g, 1)
            for c in range(8):
                st, sp = (c == 0), (c == 7)
                xo = xT_sb[:, own0 + c * 256 : own0 + (c + 1) * 256]
                xp = xT_sb[:, peer0 + c * 256 : peer0 + (c + 1) * 256]
                nc.tensor.matmul(
                    out=qk_ps[:],
                    lhsT=wqkk_sb[:, c, 0:128],
                    rhs=xo,
                    start=st,
                    stop=sp,
                    skip_group_check=True,
                )
                # qp (col group 0) || vv (col group 1) run concurrently
                nc.tensor.matmul(
                    out=pv2_ps[0:64, 0:256],
                    lhsT=wqkk_sb[:, c, 0:64],
                    rhs=xp,
                    start=st,
                    stop=sp,
                    skip_group_check=True,
                )
                nc.tensor.matmul(
                    out=pv2_ps[64:128, 256:512],
                    lhsT=wv_sb[:, c, :],
                    rhs=xo,
                    start=st,
                    stop=sp,
                    skip_group_check=True,
                )
            # evacuations: q path on vector (gates scores), k path on gpsimd
            scatter2(qk_ps[0:64, :], q_sb[0:64, :], g * 512)
            scatter2(pv2_ps[0:64, 0:256], q_sb[0:64, :], g * 512 + 128)
            nc.vector.tensor_copy(
                q_sb[64:128, g * 512 : (g + 1) * 512],
                q_sb[0:64, g * 512 : (g + 1) * 512],
            )
            # psum col block 0 = own tile 2g+1 (odd -> h64 half), block 1 =
            # own tile 2g (even -> h0 half)
            nc.vector.tensor_copy(
                k_sb[0:64, g * 128 : (g + 1) * 128], qk_ps[64:128, 128:256]
            )
            nc.vector.tensor_copy(
                k_sb[64:128, g * 128 : (g + 1) * 128], qk_ps[64:128, 0:128]
            )
            nc.vector.tensor_copy(
                vT_sb[:, g * 256 : (g + 1) * 256], pv2_ps[64:128, 256:512]
            )

        def transpose_v(j):
            vt_ps = ps_qk.tile([128, H], bf16, tag="vt", name="vt_ps")
            nc.tensor.transpose(
                vt_ps[:, 0:H],
                vT_sb[:, (j ^ 1) * 128 : ((j ^ 1) + 1) * 128],
                ident[0:64, 0:64],
            )
            nc.vector.tensor_copy(v_sb[:, j, 0:H], vt_ps[:, 0:H])

        def scores_pair(qt, i, wei):
                je, jo = 2 * i, 2 * i + 1
                last = jo == 2 * qt + 1
                wo = 256 if last else 512
                pair_ps = ps_pair.tile([128, 1024], fp32, tag="pair", name="pair_ps")
                nc.tensor.matmul(
                    out=pair_ps[:, 0:512],
                    lhsT=k_sb[0:64, i * 128 : (i + 1) * 128],
                    rhs=q_sb[0:64, qt * 512 : qt * 512 + 512],
                    start=True,
                    stop=(not last),
                    skip_group_check=True,
                )
                nc.tensor.matmul(
                    out=pair_ps[:, 512 : 512 + wo],
                    lhsT=k_sb[64:128, i * 128 : (i + 1) * 128],
                    rhs=q_sb[64:128, qt * 512 : qt * 512 + wo],
                    start=True,
                    stop=(not last),
                    skip_group_check=True,
                )
                if last:
                    # stair masks: accumulate -30/SCALE into the masked
                    # score regions pre-exp (I.T @ maskneg adds it); one
                    # N=512 mm covers even-slot stair + odd-slot stair
                    nc.tensor.matmul(
                        out=pair_ps[:, 256:768],
                        lhsT=ident[:],
                        rhs=maskneg_sb[:],
                        start=False,
                        stop=True,
                        skip_group_check=True,
                    )
                nc.scalar.activation(
                    out=wei[:, je * 512 : je * 512 + 512 + wo],
                    in_=pair_ps[:, 0 : 512 + wo],
                    func=mybir.ActivationFunctionType.Exp,
                    scale=SCALE,
                )

        def pv_out(qt, wei):
            pv = ps_pv.tile([H + 1, 512], fp32, tag="pv", name="pv_ps")
            nslots = 2 * qt + 2
            for j in range(nslots):
                w = 256 if j == nslots - 1 else 512
                nc.tensor.matmul(
                    out=pv[:, 0:w],
                    lhsT=v_sb[:, j, :],
                    rhs=wei[:, j * 512 : j * 512 + w],
                    start=(j == 0),
                    stop=(j == nslots - 1),
                    skip_group_check=True,
                )
            out_t = outpool.tile([H + 1, 512], fp32, tag="out_t")
            nc.vector.tensor_copy(out_t[:], pv[:])
            nc.sync.dma_start(out=out_ext[:, qt * 512 : (qt + 1) * 512], in_=out_t[:])

        # per-group pipeline: PV(g-1) + v transposes fill the PE while the
        # vector engine evacuates proj(g) (whose q/k gate scores(g)); the
        # exp chain runs one group behind on ACT.
        for g in range(QT):
            proj_group(g)
            if g >= 1:
                pv_out(g - 1, weis[g - 1])
            transpose_v(2 * g)
            transpose_v(2 * g + 1)
            for i in range(g + 1):
                scores_pair(g, i, weis[g])
        pv_out(QT - 1, weis[QT - 1])

    nc.compile()
    return nc


def _own_rows(p):
    """x column order for parity p: own tiles PAIR-SWAPPED [o1,o0,o3,o2,...]
    so the q psum scatters to local order with a single strided copy."""
    order = [1, 0, 3, 2, 5, 4, 7, 6]
    return np.concatenate(
        [np.arange((2 * j + p) * 128, (2 * j + p) * 128 + 128) for j in order]
    )


def _local_q_perm(p):
    perm = np.empty(T, dtype=np.int64)
    for qt in range(QT):
        tiles = [4 * qt + 2 + p, 4 * qt + 3 - p, 4 * qt + p, 4 * qt + 1 - p]
        for ci, g in enumerate(tiles):
            lo = qt * 512 + ci * 128
            perm[lo : lo + 128] = np.arange(g * 128, g * 128 + 128)
    return perm


def _make_in_maps(x, Wq, Wk, Wv):
    bf = ml_dtypes.bfloat16
    wqkk = np.concatenate([Wq, Wk], axis=1)  # [C, 128]
    wqkk_pre = np.ascontiguousarray(
        wqkk.reshape(8, 128, 128).transpose(1, 0, 2).astype(bf)
    )
    wv_pre = np.ascontiguousarray(
        Wv.reshape(8, 128, 64).transpose(1, 0, 2).astype(bf)
    )
    tri = (np.arange(128)[:, None] <= np.arange(128)[None, :]).astype(np.float32)
    in_maps = []
    for c in range(8):
        b, p = c // 2, c % 2
        xT_pre = np.empty((128, 16384), dtype=bf)
        for h, rows in ((0, _own_rows(p)), (1, _own_rows(1 - p))):
            for g in range(4):
                cols = rows[g * 256 : (g + 1) * 256]
                blkT = x[b][cols, :].T  # [1024, 256]
                base = (2 * g + h) * 2048
                xT_pre[:, base : base + 2048] = (
                    blkT.reshape(8, 128, 256).transpose(1, 0, 2).reshape(128, 2048)
                ).astype(bf)
        X = np.ones((128, 128), np.float32) if p == 0 else np.zeros((128, 128), np.float32)
        m256 = (1.0 - np.concatenate([tri, X], axis=1)) * (-30.0 / SCALE)
        mneg = np.concatenate([m256, m256], axis=1)  # [128, 512]
        in_maps.append(
            {"xT": xT_pre, "wqkk": wqkk_pre, "wv": wv_pre,
             "maskneg": np.ascontiguousarray(mneg.astype(bf))}
        )
    return in_maps


def _combine(per_core_out):
    out = np.empty((B, T, H), dtype=np.float32)
    for b in range(B):
        S = None
        for p in range(2):
            P_local = np.asarray(per_core_out[2 * b + p], dtype=np.float32)
            perm = _local_q_perm(p)
            P_glob = np.empty_like(P_local)
            P_glob[:, perm] = P_local
            S = P_glob if S is None else S + P_glob
        out[b] = (S[0:H, :] / S[H : H + 1, :]).T
    return out


def kernel(x, Wq, Wk, Wv):
    global _COMPILED, LAST_EXEC_NS, LAST_RESULTS
    from concourse.bass_utils import run_bass_kernel_spmd

    x = np.ascontiguousarray(np.asarray(x, dtype=np.float32))
    Wq = np.asarray(Wq, dtype=np.float32)
    Wk = np.asarray(Wk, dtype=np.float32)
    Wv = np.asarray(Wv, dtype=np.float32)

    if _COMPILED is None:
        _COMPILED = _build_nc()
    nc = _COMPILED

    in_maps = _make_in_maps(x, Wq, Wk, Wv)
    trace = os.environ.get("BASS_KERNEL_TRACE", "0") == "1"
    res = run_bass_kernel_spmd(nc, in_maps, core_ids=list(range(8)), trace=trace)
    LAST_EXEC_NS = getattr(res, "exec_time_ns", None)
    LAST_RESULTS = res
    return _combine([res.results[c]["out"] for c in range(8)])


# revision 17
# speedup vs baseline: 1.0788x; 1.0788x over previous
"""Single-head causal attention (B=4, T=2048, C=1024, H=64) on 8 TRN2 NeuronCores.

Sharding: batch b -> core pair (2b, 2b+1); core parity p owns interleaved
128-row key tiles {2m+p}.  Each core projects q for ALL 2048 queries and k,v
for its own 1024 keys, computes causal scores^T -> exp -> stair mask ->
wei@[v|1] partials for all queries vs its own keys.  Host adds pair partials
and normalizes (denominator = ones-column of the augmented v matmul).

Query-GROUP pipelined schedule: x arrives in 4 column groups (one per local
512-query tile, own+peer halves); per group g the PE runs projections ->
scores(g) while the scalar engine exps the previous pairs and PV(g-1)
interleaves.  The exp chain (the serial ACT bottleneck, ~8.5us) starts as
soon as group 0 is projected instead of after the full 4.4MB x load.

Data layout (all bf16, host pre-cast):
 - xT dram [128, 16384]: 8 slabs of 2048 cols, slab (g,h) = group g,
   h=0 own / h=1 peer query columns in pair-swapped order [t(2g+1)|t(2g)],
   chunk-major over C (8 chunks of 128 C-rows x 256 cols).
 - per group g, chunk c: qk mm (M=128, [wq|wk]) over own cols; qp (M=64,
   col group 0) and vv (M=64, col group 1) run CONCURRENTLY as column-tiled
   mms over peer / own cols.
 - q_sb local col order per 512 block: [own-odd | peer-odd | own-even |
   peer-even]; k_sb / vT_sb storage slot s holds own tile s^1.
 - k (and q) duplicated in both 64-row partition halves so the two
   row-tiled score mms of a slot pair run concurrently.
 - causal trimming + stair masks identical to the monolithic version.
"""

import os
import sys

sys.path.insert(0, "/opt/trn_rl_repo")

import numpy as np
import ml_dtypes

B, T, C, H = 4, 2048, 1024, 64
QT = 4
SCALE = float(C) ** -0.5
NWARM = 4

_COMPILED = None
LAST_EXEC_NS = None
LAST_RESULTS = None


def _build_nc():
    import concourse.bass as bass_mod
    import concourse.mybir as mybir
    import concourse.tile as tile
    from concourse import bacc
    from contextlib import ExitStack

    fp32 = mybir.dt.float32
    bf16 = mybir.dt.bfloat16
    fp8 = mybir.dt.float8e4

    nc = bacc.Bacc(
        "TRN2",
        target_bir_lowering=False,
        debug=False,
        num_devices=8,
        detect_race_conditions=True,
    )
    xT = nc.declare_dram_parameter("xT", [128, 16384], bf16, isOutput=False)
    wqkk = nc.declare_dram_parameter("wqkk", [128, 8, 128], bf16, isOutput=False)
    wv = nc.declare_dram_parameter("wv", [128, 8, 64], bf16, isOutput=False)
    maskneg = nc.declare_dram_parameter("maskneg", [128, 512], bf16, isOutput=False)
    out_ext = nc.declare_dram_parameter("out", [H + 1, T], fp32, isOutput=True)
    junkd = nc.declare_dram_parameter("junkd", [128, 16], bf16, isOutput=True)

    with ExitStack() as ctx:
        tc = ctx.enter_context(tile.TileContext(nc))
        persist = ctx.enter_context(tc.tile_pool(name="persist", bufs=1))
        weipool = ctx.enter_context(tc.tile_pool(name="wei", bufs=3))
        outpool = ctx.enter_context(tc.tile_pool(name="outp", bufs=2))

        xT_sb = persist.tile([128, 16384], bf16, tag="xT_sb")
        wqkk_sb = persist.tile([128, 8, 128], bf16, tag="wqkk_sb")
        wv_sb = persist.tile([128, 8, 64], bf16, tag="wv_sb")
        maskneg_sb = persist.tile([128, 512], bf16, tag="maskneg_sb")
        q_sb = persist.tile([128, T], bf16, tag="q_sb")
        k_sb = persist.tile([128, 512], bf16, tag="k_sb")
        vT_sb = persist.tile([64, 1024], bf16, tag="vT_sb")
        v_sb = persist.tile([128, 8, H + 1], bf16, tag="v_sb")
        ident = persist.tile([128, 128], bf16, tag="ident")
        scratch = persist.tile([128, 512], bf16, tag="scratch")
        warm_tok = persist.tile([1, 8], fp32, tag="warm_tok")
        act_tok = persist.tile([1, 8], bf16, tag="act_tok")
        junk_sb = persist.tile([128, 16], bf16, tag="junk_sb")

        # ---- gpsimd: constants first (they gate the PE warmup), DMAs none.
        nc.gpsimd.memset(scratch[:], 0.0)
        nc.gpsimd.memset(ident[:], 0.0)
        nc.gpsimd.affine_select(
            out=ident[:],
            in_=ident[:],
            compare_op=mybir.AluOpType.not_equal,
            fill=1.0,
            base=0,
            pattern=[[-1, 128]],
            channel_multiplier=1,
        )
        nc.gpsimd.memset(v_sb[:, :, H : H + 1], 1.0)

        weis = [
            weipool.tile([128, (2 * g + 2) * 512], bf16, tag="wei", name=f"wei{g}")
            for g in range(QT)
        ]

        # ---- loads: all of x on the sync HWDGE ring in consumption order
        # (group-major, own/peer chunk-halves interleaved so proj(g) can
        # start on chunks 0-3); weights on scalar (land first, then the
        # scalar queue is free for the exp chain).
        def base(g, c):
            return g * 4096 + c * 512

        # x: a DMA ring round-robins packets across ALL queued transfers
        # (they finish together), so sequential group arrival requires
        # chaining each issue on the previous group's landing / on exp
        # progress (the serial bottleneck).  4 sub-transfers per slab keep
        # ~8 engines busy (~280 GB/s) while exactly one group is in flight.
        # sub-transfer s of group g covers chunks 2s,2s+1 ([128, 1024]);
        # subs 0,2 ride the gpsimd ring, subs 1,3 the sync ring.  Group
        # g+1's issue fires when g's first sub on that ring has landed
        # (half-overlap: no ring dead-air, still near-in-order arrival).
        def sub_ap(g, s):
            b0 = base(g, 0) + s * 1024
            return xT_sb[:, b0 : b0 + 1024], xT[:, b0 : b0 + 1024]

        def sub_tail(g, s):
            t = xT_sb[:, base(g, 0) + s * 1024 + 1023 : base(g, 0) + s * 1024 + 1024]
            return t

        def x_issue(ring, g, subs):
            for s in subs:
                o, i = sub_ap(g, s)
                ring.dma_start(out=o, in_=i)

        nc.gpsimd.dma_start(out=wqkk_sb[:], in_=wqkk[:])
        nc.sync.dma_start(out=wv_sb[:], in_=wv[:])
        x_issue(nc.gpsimd, 0, (0, 2))
        x_issue(nc.sync, 0, (1, 3))
        nc.sync.dma_start(out=maskneg_sb[:], in_=maskneg[:])
        for g in range(1, 4):
            nc.gpsimd.tensor_copy(junk_sb[:, g : g + 1], sub_tail(g - 1, 0))
            nc.sync.dma_start(out=junkd[:, g : g + 1], in_=sub_tail(g - 1, 1))
            x_issue(nc.gpsimd, g, (0, 2))
            x_issue(nc.sync, g, (1, 3))

        # ---- PE pre-warm (gated only on gpsimd constants) + Exp spline
        # table preload on ACT.
        with tc.tile_pool(name="ps_warm", bufs=1, space="PSUM") as ps_warm:
            wps = ps_warm.tile([128, 512], fp32, tag="warm", name="warm_ps")
            for i in range(NWARM):
                nc.tensor.matmul(
                    out=wps[:],
                    lhsT=ident[:],
                    rhs=scratch[:],
                    start=(i == 0),
                    stop=(i == NWARM - 1),
                    skip_group_check=True,
                )
            nc.vector.tensor_copy(warm_tok[0:1, 0:8], wps[0:1, 0:8])
        nc.scalar.activation(
            out=act_tok[0:1, 0:8],
            in_=scratch[0:1, 0:8],
            func=mybir.ActivationFunctionType.Exp,
        )

        # ---- per-group projections ----
        ps_qk = ctx.enter_context(tc.tile_pool(name="ps_qk", bufs=1, space="PSUM"))
        ps_pv2 = ctx.enter_context(tc.tile_pool(name="ps_pv2", bufs=1, space="PSUM"))
        ps_pair = ctx.enter_context(tc.tile_pool(name="ps_pair", bufs=2, space="PSUM"))
        ps_pv = ctx.enter_context(tc.tile_pool(name="ps_pv", bufs=1, space="PSUM"))

        def own_ap(g, c):
            # own cols of chunk c: blocks 0 (o1) and 2 (o0) of the 512
            s = xT_sb[:, base(g, c) : base(g, c) + 1]
            return bass_mod.AP(
                tensor=s.tensor, offset=s.offset, ap=[s.ap[0], [256, 2], [1, 128]]
            )

        def proj_group(g):
            # dense N=512 qk stream + N=256 v stream: near-100% PE duty so
            # the HAM clock gate opens (2.4 GHz) and LDWs hide behind mms
            qk_ps = ps_qk.tile([128, 512], fp32, tag="qk", name=f"qk{g}")
            vv_ps = ps_pv2.tile([64, 256], fp32, tag="vv", name=f"vv{g}")
            for c in range(8):
                st, sp = (c == 0), (c == 7)
                nc.tensor.matmul(
                    out=qk_ps[:],
                    lhsT=wqkk_sb[:, c, 0:128],
                    rhs=xT_sb[:, base(g, c) : base(g, c) + 512],
                    start=st,
                    stop=sp,
                    skip_group_check=True,
                )
                nc.tensor.matmul(
                    out=vv_ps[:],
                    lhsT=wv_sb[:, c, :],
                    rhs=own_ap(g, c),
                    start=st,
                    stop=sp,
                    skip_group_check=True,
                )
            # evacuations (vector): q contiguous + dup; k split by tile
            # parity (psum block 0 = own tile 2g+1 -> h64, block 2 = own
            # tile 2g -> h0); vT in storage-slot order [2g, 2g+1]
            nc.vector.tensor_copy(
                q_sb[0:64, g * 512 : (g + 1) * 512], qk_ps[0:64, :]
            )
            nc.vector.tensor_copy(
                q_sb[64:128, g * 512 : (g + 1) * 512],
                q_sb[0:64, g * 512 : (g + 1) * 512],
            )
            nc.vector.tensor_copy(
                k_sb[64:128, g * 128 : (g + 1) * 128], qk_ps[64:128, 0:128]
            )
            nc.vector.tensor_copy(
                k_sb[0:64, g * 128 : (g + 1) * 128], qk_ps[64:128, 256:384]
            )
            nc.vector.tensor_copy(
                vT_sb[:, g * 256 : (g + 1) * 256], vv_ps[:]
            )

        def transpose_v(j):
            vt_ps = ps_qk.tile([128, H], bf16, tag="vt", name="vt_ps")
            nc.tensor.transpose(
                vt_ps[:, 0:H],
                vT_sb[:, (j ^ 1) * 128 : ((j ^ 1) + 1) * 128],
                ident[0:64, 0:64],
            )
            nc.vector.tensor_copy(v_sb[:, j, 0:H], vt_ps[:, 0:H])

        def scores_pair(qt, i, wei):
                je, jo = 2 * i, 2 * i + 1
                last = jo == 2 * qt + 1
                wo = 256 if last else 512
                pair_ps = ps_pair.tile([128, 1024], fp32, tag="pair", name="pair_ps")
                nc.tensor.matmul(
                    out=pair_ps[:, 0:512],
                    lhsT=k_sb[0:64, i * 128 : (i + 1) * 128],
                    rhs=q_sb[0:64, qt * 512 : qt * 512 + 512],
                    start=True,
                    stop=(not last),
                    skip_group_check=True,
                )
                nc.tensor.matmul(
                    out=pair_ps[:, 512 : 512 + wo],
                    lhsT=k_sb[64:128, i * 128 : (i + 1) * 128],
                    rhs=q_sb[64:128, qt * 512 : qt * 512 + wo],
                    start=True,
                    stop=(not last),
                    skip_group_check=True,
                )
                if last:
                    # stair masks: accumulate -30/SCALE into the masked
                    # score regions pre-exp (I.T @ maskneg adds it); one
                    # N=512 mm covers even-slot stair + odd-slot stair
                    nc.tensor.matmul(
                        out=pair_ps[:, 256:768],
                        lhsT=ident[:],
                        rhs=maskneg_sb[:],
                        start=False,
                        stop=True,
                        skip_group_check=True,
                    )
                nc.scalar.activation(
                    out=wei[:, je * 512 : je * 512 + 512 + wo],
                    in_=pair_ps[:, 0 : 512 + wo],
                    func=mybir.ActivationFunctionType.Exp,
                    scale=SCALE,
                )

        def pv_out(qt, wei):
            pv = ps_pv.tile([H + 1, 512], fp32, tag="pv", name="pv_ps")
            nslots = 2 * qt + 2
            for j in range(nslots):
                w = 256 if j == nslots - 1 else 512
                nc.tensor.matmul(
                    out=pv[:, 0:w],
                    lhsT=v_sb[:, j, :],
                    rhs=wei[:, j * 512 : j * 512 + w],
                    start=(j == 0),
                    stop=(j == nslots - 1),
                    skip_group_check=True,
                )
            out_t = outpool.tile([H + 1, 512], fp32, tag="out_t")
            nc.vector.tensor_copy(out_t[:], pv[:])
            nc.sync.dma_start(out=out_ext[:, qt * 512 : (qt + 1) * 512], in_=out_t[:])

        # per-group pipeline: PV(g-1) + v transposes fill the PE while the
        # vector engine evacuates proj(g) (whose q/k gate scores(g)); the
        # exp chain runs one group behind on ACT.
        for g in range(QT):
            proj_group(g)
            if g >= 1:
                pv_out(g - 1, weis[g - 1])
            transpose_v(2 * g)
            transpose_v(2 * g + 1)
            for i in range(g + 1):
                scores_pair(g, i, weis[g])
        pv_out(QT - 1, weis[QT - 1])

    nc.compile()
    return nc


def _own_rows(p):
    """x column order for parity p: own tiles PAIR-SWAPPED [o1,o0,o3,o2,...]
    so the q psum scatters to local order with a single strided copy."""
    order = [1, 0, 3, 2, 5, 4, 7, 6]
    return np.concatenate(
        [np.arange((2 * j + p) * 128, (2 * j + p) * 128 + 128) for j in order]
    )


def _local_q_perm(p):
    perm = np.empty(T, dtype=np.int64)
    for qt in range(QT):
        tiles = [4 * qt + 2 + p, 4 * qt + 3 - p, 4 * qt + p, 4 * qt + 1 - p]
        for ci, g in enumerate(tiles):
            lo = qt * 512 + ci * 128
            perm[lo : lo + 128] = np.arange(g * 128, g * 128 + 128)
    return perm


def _make_in_maps(x, Wq, Wk, Wv):
    bf = ml_dtypes.bfloat16
    wqkk = np.concatenate([Wq, Wk], axis=1)  # [C, 128]
    wqkk_pre = np.ascontiguousarray(
        wqkk.reshape(8, 128, 128).transpose(1, 0, 2).astype(bf)
    )
    wv_pre = np.ascontiguousarray(
        Wv.reshape(8, 128, 64).transpose(1, 0, 2).astype(bf)
    )
    tri = (np.arange(128)[:, None] <= np.arange(128)[None, :]).astype(np.float32)
    in_maps = []
    for c in range(8):
        b, p = c // 2, c % 2
        xT_pre = np.empty((128, 16384), dtype=bf)
        ro, rp = _own_rows(p), _own_rows(1 - p)
        for g in range(4):
            rows4 = np.concatenate(
                [
                    ro[g * 256 : g * 256 + 128],       # own 2g+1
                    rp[g * 256 : g * 256 + 128],       # peer 2g+1
                    ro[g * 256 + 128 : (g + 1) * 256], # own 2g
                    rp[g * 256 + 128 : (g + 1) * 256], # peer 2g
                ]
            )
            blkT = x[b][rows4, :].T  # [1024, 512]
            xT_pre[:, g * 4096 : (g + 1) * 4096] = (
                blkT.reshape(8, 128, 512).transpose(1, 0, 2).reshape(128, 4096)
            ).astype(bf)
        X = np.ones((128, 128), np.float32) if p == 0 else np.zeros((128, 128), np.float32)
        m256 = (1.0 - np.concatenate([tri, X], axis=1)) * (-30.0 / SCALE)
        mneg = np.concatenate([m256, m256], axis=1)  # [128, 512]
        in_maps.append(
            {"xT": xT_pre, "wqkk": wqkk_pre, "wv": wv_pre,
             "maskneg": np.ascontiguousarray(mneg.astype(bf))}
        )
    return in_maps


def _combine(per_core_out):
    out = np.empty((B, T, H), dtype=np.float32)
    for b in range(B):
        S = None
        for p in range(2):
            P_local = np.asarray(per_core_out[2 * b + p], dtype=np.float32)
            perm = _local_q_perm(p)
            P_glob = np.empty_like(P_local)
            P_glob[:, perm] = P_local
            S = P_glob if S is None else S + P_glob
        out[b] = (S[0:H, :] / S[H : H + 1, :]).T
    return out


def kernel(x, Wq, Wk, Wv):
    global _COMPILED, LAST_EXEC_NS, LAST_RESULTS
    from concourse.bass_utils import run_bass_kernel_spmd

    x = np.ascontiguousarray(np.asarray(x, dtype=np.float32))
    Wq = np.asarray(Wq, dtype=np.float32)
    Wk = np.asarray(Wk, dtype=np.float32)
    Wv = np.asarray(Wv, dtype=np.float32)

    if _COMPILED is None:
        _COMPILED = _build_nc()
    nc = _COMPILED

    in_maps = _make_in_maps(x, Wq, Wk, Wv)
    trace = os.environ.get("BASS_KERNEL_TRACE", "0") == "1"
    res = run_bass_kernel_spmd(nc, in_maps, core_ids=list(range(8)), trace=trace)
    LAST_EXEC_NS = getattr(res, "exec_time_ns", None)
    LAST_RESULTS = res
    return _combine([res.results[c]["out"] for c in range(8)])
